# revision 1
# baseline (speedup 1.0000x reference)
"""Trainium2 Bass kernel for nn_CRATE (12-layer CRATE-style transformer).

Sharding over 8 NeuronCores: 4 batch groups x 2-way parity-interleaved
sequence split.  Core c handles batch b=c//2 and parity half=c%2: it owns
absolute rows {2*j + half, j=0..511}.  With this split both halves have an
IDENTICAL causal block structure (local q-tile lt attends exactly lt+1
128-blocks of each rank's keys), so a single SPMD program serves all
cores; every half-dependence (rope phases, diagonal masks, embedding rows)
is per-core input data.  Per layer the tied-QKV tensor w (post rope +
qk-norm, bf16) is exchanged inside each pair with an AllGather; all other
compute is local.  lm_head: each core does full vocab for its 512 rows.
Matmuls are bf16 with fp32 accumulation; residual stream, norms, softmax
statistics stay fp32.
"""

import sys

sys.path.insert(0, "/opt/trn_rl_repo")

import numpy as np
import ml_dtypes

BF16 = ml_dtypes.bfloat16

B, T = 4, 1024
V, E, L, H = 50304, 768, 12, 6
HD = 128
HID = 3072
EPS = 1e-6
ROPE_BASE = 10000.0
SCALE = HD ** -0.5
N_CORES = 8
TQ = 512            # rows per core
NT = 4              # 128-row tiles per core
NE = 6              # 128-col tiles of E
NJT = 24            # 128-col tiles of HID
NJC = 6             # 512-wide chunks of HID
NEG = -1e10
VCH = [(s, min(512, V - s)) for s in range(0, V, 512)]   # 99 vocab chunks
import os as _os
L_RUN = int(_os.environ.get("KBENCH_LAYERS", str(L)))


def _rope_tables():
    ch = np.arange(0, HD, 2, dtype=np.float32)
    inv = (1.0 / (ROPE_BASE ** (ch / np.float32(HD)))).astype(np.float32)
    t = np.arange(T, dtype=np.float32)
    fr = np.outer(t, inv).astype(np.float32)
    return np.cos(fr).astype(np.float32), np.sin(fr).astype(np.float32)


def _own_rows(half):
    return 2 * np.arange(TQ) + half


def _f32(a):
    return np.asarray(a, dtype=np.float32)


def _bf(a):
    return np.asarray(a).astype(BF16)


def _bfr(a):
    return np.asarray(a).astype(BF16).astype(np.float32)


def _diag_masks(half):
    """dmask[r][qi,ki] = 0 where (2qi+half) >= (2ki+r) else NEG."""
    qi = np.arange(128)[:, None]
    ki = np.arange(128)[None, :]
    out = np.empty((2, 128, 128), dtype=np.float32)
    for r in range(2):
        out[r] = np.where(2 * qi + half >= 2 * ki + r, 0.0, NEG)
    return out


def _host_prep(inputs):
    idx = np.asarray(inputs["idx"])
    wte = _f32(inputs["wte"])
    prep = {}
    prep["qkvT"] = np.ascontiguousarray(
        _f32(inputs["qkv_w"]).transpose(0, 2, 1)).astype(BF16)     # [L, E, E] (e, f)
    prep["cprojT"] = np.ascontiguousarray(
        _f32(inputs["cproj_w"]).transpose(0, 2, 1)).astype(BF16)   # [L, E, E] (e, e')
    prep["dencT"] = np.ascontiguousarray(
        _f32(inputs["denc_w"]).transpose(0, 2, 1)).astype(BF16)    # [L, E, HID]
    prep["ddecT"] = np.ascontiguousarray(
        _f32(inputs["ddec_w"]).transpose(0, 2, 1)).astype(BF16)    # [L, HID, E]
    prep["lmT"] = np.ascontiguousarray(_f32(inputs["lm_head_w"]).T).astype(BF16)
    thr = _f32(inputs["thr"])
    prep["thrneg"] = np.ascontiguousarray(
        (-thr).reshape(L, NJT, 128).transpose(2, 0, 1)).astype(np.float32)
    prep["lamr"] = np.ascontiguousarray(
        np.broadcast_to(_f32(inputs["resid_lambdas"]), (128, L))).astype(np.float32)
    prep["lamx"] = np.ascontiguousarray(
        np.broadcast_to(_f32(inputs["x0_lambdas"]), (128, L))).astype(np.float32)

    cos, sin = _rope_tables()          # [T, 64]
    per_core = []
    for c in range(N_CORES):
        b, half = c // 2, c % 2
        rows = _own_rows(half)
        pc = {}
        pc["xemb"] = np.ascontiguousarray(wte[idx[b][rows]]).astype(np.float32)
        pc["cosr"] = np.ascontiguousarray(np.tile(cos[rows], (1, H))).astype(np.float32)
        pc["sinr"] = np.ascontiguousarray(np.tile(sin[rows], (1, H))).astype(np.float32)
        pc["dmask"] = _diag_masks(half)
        per_core.append(pc)
    return prep, per_core


# --------------------------------------------------------------------------
# numpy mirror of the exact device dataflow (bf16 casts in the same places)
# --------------------------------------------------------------------------

def _mirror_pair(prep, pcs):
    xs = []
    for half in range(2):
        xe = pcs[half]["xemb"]
        r = 1.0 / np.sqrt((xe * xe).sum(-1, keepdims=True) / E + EPS)
        xs.append((xe * r).astype(np.float32))
    x0s = [x.copy() for x in xs]

    for i in range(L_RUN):
        rl = prep["lamr"][0, i]
        xl = prep["lamx"][0, i]
        w_bfs = []
        for half in range(2):
            x = (xs[half] * rl + x0s[half] * xl).astype(np.float32)
            xs[half] = x
            r = 1.0 / np.sqrt((x * x).sum(-1, keepdims=True) / E + EPS)
            h_bf = _bfr(x * r)
            w_raw = h_bf @ _bfr(prep["qkvT"][i])          # [TQ, E]
            wh = w_raw.reshape(TQ, H, HD)
            rw = 1.0 / np.sqrt((wh * wh).sum(-1, keepdims=True) / HD + EPS)
            cosr = pcs[half]["cosr"].reshape(TQ, H, 64)
            sinr = pcs[half]["sinr"].reshape(TQ, H, 64)
            x1, x2 = wh[..., :64], wh[..., 64:]
            wn = np.concatenate(
                [x1 * cosr + x2 * sinr, x2 * cosr - x1 * sinr], axis=-1)
            w_bfs.append(_bf((wn * rw).reshape(TQ, E)))
        # AllGather result, rank-major rows, viewed [r, k_local, h, d]
        wall = np.stack([w.astype(np.float32).reshape(TQ, H, HD)
                         for w in w_bfs])

        new_xs = []
        for half in range(2):
            x = xs[half]
            dmask = pcs[half]["dmask"]
            oT = np.zeros((H, HD, TQ), dtype=np.float32)
            for h in range(H):
                for lt in range(NT):
                    nk = (lt + 1) * 128
                    q = wall[half, lt * 128:(lt + 1) * 128, h]     # [128, HD]
                    srs, ps = [], []
                    for rnk in range(2):
                        s = q @ wall[rnk, :nk, h].T                # [128, nk]
                        s[:, lt * 128:nk] += dmask[rnk]
                        srs.append(s)
                    m = np.maximum(srs[0].max(-1), srs[1].max(-1))[:, None]
                    se = 0.0
                    for rnk in range(2):
                        p = _bfr(np.exp((srs[rnk] - m) * SCALE))
                        se = se + p.sum(-1, keepdims=True)
                        ps.append(p)
                    for rnk in range(2):
                        p = _bfr(ps[rnk] * (1.0 / se))             # [128(q), nk]
                        pTfull = np.zeros((TQ, 128), dtype=np.float32)
                        pTfull[:nk] = p.T
                        oT[h, :, lt * 128:(lt + 1) * 128] += \
                            wall[rnk, :, h].T @ pTfull             # [d, 128]
            oT_bf = _bfr(oT)
            o = oT_bf.transpose(2, 0, 1).reshape(TQ, E)
            x = x + o @ _bfr(prep["cprojT"][i])
            r2 = 1.0 / np.sqrt((x * x).sum(-1, keepdims=True) / E + EPS)
            h2 = _bfr(x * r2)
            a_raw = h2 @ _bfr(prep["dencT"][i])
            thr_i = -prep["thrneg"][:, i, :].T.reshape(HID)
            aT = _bfr(np.maximum(a_raw - thr_i, 0.0))
            x = x + aT @ _bfr(prep["ddecT"][i])
            new_xs.append(x.astype(np.float32))
        xs = new_xs

    outs = []
    for half in range(2):
        x = xs[half]
        r = 1.0 / np.sqrt((x * x).sum(-1, keepdims=True) / E + EPS)
        outs.append(_bfr(x * r) @ _bfr(prep["lmT"]))
    return outs


def kernel_numpy(**inputs):
    prep, per_core = _host_prep(inputs)
    out = np.empty((B, T, V), dtype=np.float32)
    for b in range(B):
        logits = _mirror_pair(prep, per_core[2 * b:2 * b + 2])
        for half in range(2):
            out[b, _own_rows(half)] = logits[half]
    return out


# --------------------------------------------------------------------------
# Bass/Tile kernel
# --------------------------------------------------------------------------

_NC_CACHE = None
LAST_RESULT = None


def _build_nc():
    import concourse.bacc as bacc
    import concourse.mybir as mybir
    import concourse.tile as tile
    from concourse.masks import make_identity

    f32 = mybir.dt.float32
    bf16 = mybir.dt.bfloat16
    AF = mybir.ActivationFunctionType
    ALU = mybir.AluOpType

    nc = bacc.Bacc("TRN2", target_bir_lowering=False, debug=False,
                   num_devices=N_CORES)

    d_xemb = nc.dram_tensor("xemb", [TQ, E], f32, kind="ExternalInput")
    d_cosr = nc.dram_tensor("cosr", [TQ, H * 64], f32, kind="ExternalInput")
    d_sinr = nc.dram_tensor("sinr", [TQ, H * 64], f32, kind="ExternalInput")
    d_dmask = nc.dram_tensor("dmask", [2, 128, 128], f32, kind="ExternalInput")
    d_qkvT = nc.dram_tensor("qkvT", [L, E, E], bf16, kind="ExternalInput")
    d_cprojT = nc.dram_tensor("cprojT", [L, E, E], bf16, kind="ExternalInput")
    d_dencT = nc.dram_tensor("dencT", [L, E, HID], bf16, kind="ExternalInput")
    d_ddecT = nc.dram_tensor("ddecT", [L, HID, E], bf16, kind="ExternalInput")
    d_lmT = nc.dram_tensor("lmT", [E, V], bf16, kind="ExternalInput")
    d_thrneg = nc.dram_tensor("thrneg", [128, L, NJT], f32, kind="ExternalInput")
    d_lamr = nc.dram_tensor("lamr", [128, L], f32, kind="ExternalInput")
    d_lamx = nc.dram_tensor("lamx", [128, L], f32, kind="ExternalInput")
    d_logits = nc.dram_tensor("logits", [TQ, V], f32, kind="ExternalOutput")

    groups = [[0, 1], [2, 3], [4, 5], [6, 7]]

    from contextlib import ExitStack

    with tile.TileContext(nc) as tc, ExitStack() as es:
        if True:
            st = es.enter_context(tc.tile_pool(name="state", bufs=1))
            ptp = es.enter_context(tc.tile_pool(name="ptp", bufs=1))
            dpool = es.enter_context(tc.tile_pool(name="dram", bufs=2, space="DRAM"))
            psA = es.enter_context(tc.tile_pool(name="psA", bufs=3, space="PSUM"))
            psB = es.enter_context(tc.tile_pool(name="psB", bufs=3, space="PSUM"))
            psT = es.enter_context(tc.tile_pool(name="psT", bufs=2, space="PSUM"))
            # ---------------- persistent state ----------------
            ident = st.tile([128, 128], f32, tag="ident")
            make_identity(nc, ident[:])
            ident_bf = st.tile([128, 128], bf16, tag="ident_bf")
            nc.vector.tensor_copy(ident_bf[:], ident[:])
            dmask_t = st.tile([128, 2, 128], f32, tag="dmask")
            nc.sync.dma_start(dmask_t[:], d_dmask[:].rearrange("r q k -> q r k"))

            eps_t = st.tile([128, 1], f32, tag="eps")
            nc.vector.memset(eps_t[:], EPS)
            lamr_t = st.tile([128, L], f32, tag="lamr")
            lamx_t = st.tile([128, L], f32, tag="lamx")
            thrneg_t = st.tile([128, L, NJT], f32, tag="thrneg")
            nc.sync.dma_start(lamr_t[:], d_lamr[:])
            nc.sync.dma_start(lamx_t[:], d_lamx[:])
            nc.sync.dma_start(thrneg_t[:], d_thrneg[:])

            x_t = [st.tile([128, E], f32, tag=f"x{t}", name=f"x_{t}") for t in range(NT)]
            x0_t = [st.tile([128, E], f32, tag=f"x0{t}", name=f"x0_{t}") for t in range(NT)]
            cosr_t = [st.tile([128, H * 64], f32, tag=f"cos{t}", name=f"cosr_{t}") for t in range(NT)]
            sinr_t = [st.tile([128, H * 64], f32, tag=f"sin{t}", name=f"sinr_{t}") for t in range(NT)]
            for t in range(NT):
                nc.sync.dma_start(cosr_t[t][:], d_cosr[t * 128:(t + 1) * 128, :])
                nc.sync.dma_start(sinr_t[t][:], d_sinr[t * 128:(t + 1) * 128, :])
            hfT = [st.tile([128, TQ], bf16, tag=f"hfT{e}", name=f"hfT_{e}") for e in range(NE)]

            # pT tiles: zeroed once; transposes rewrite exactly the same
            # valid region every reuse, zeros elsewhere persist.
            pT_tiles = [ptp.tile([128, 2, NT, TQ], bf16, tag=f"pT{z}", name=f"pT_{z}")
                        for z in range(2)]
            for pt in pT_tiles:
                nc.vector.memset(pt[:], 0.0)

            def rmsnorm_stats(src_tiles, dim, sm, junk_pool):
                rs = []
                for t in range(NT):
                    junk = junk_pool.tile([128, dim], f32, tag="sc")
                    ssq = sm.tile([128, 1], f32, tag="nss")
                    nc.scalar.activation(junk[:], src_tiles[t][:], AF.Square,
                                         accum_out=ssq[:])
                    sq = sm.tile([128, 1], f32, tag="nsq")
                    nc.scalar.activation(sq[:], ssq[:], AF.Sqrt,
                                         bias=eps_t[:], scale=1.0 / dim)
                    r = sm.tile([128, 1], f32, tag="nr")
                    nc.vector.reciprocal(r[:], sq[:])
                    rs.append(r)
                return rs

            with ExitStack() as les:
                wq_p = les.enter_context(tc.tile_pool(name="wq", bufs=NE))
                wc_p = les.enter_context(tc.tile_pool(name="wc", bufs=NE))
                wd_p = les.enter_context(tc.tile_pool(name="wd", bufs=12))
                wdd_p = les.enter_context(tc.tile_pool(name="wdd", bufs=6))
                sc_p = les.enter_context(tc.tile_pool(name="sc", bufs=5))
                rp_p = les.enter_context(tc.tile_pool(name="rp", bufs=2))
                hh_p = les.enter_context(tc.tile_pool(name="hh", bufs=4))
                ht_p = les.enter_context(tc.tile_pool(name="ht", bufs=7))
                sm_p = les.enter_context(tc.tile_pool(name="sm", bufs=12))
                wb_p = les.enter_context(tc.tile_pool(name="wb", bufs=4))
                wo_p = les.enter_context(tc.tile_pool(name="wo", bufs=6))
                wl_p = les.enter_context(tc.tile_pool(name="wl", bufs=3))
                wt_p = les.enter_context(tc.tile_pool(name="wt", bufs=3))
                pp_p = les.enter_context(tc.tile_pool(name="pp", bufs=3))
                ot_p = les.enter_context(tc.tile_pool(name="ot", bufs=6))
                at_p = les.enter_context(tc.tile_pool(name="at", bufs=6))
                # ---------------- embedding + initial rmsnorm ----------------
                xe_tiles = []
                for t in range(NT):
                    xe = sc_p.tile([128, E], f32, tag="sc")
                    nc.sync.dma_start(xe[:], d_xemb[t * 128:(t + 1) * 128, :])
                    xe_tiles.append(xe)
                r_emb = rmsnorm_stats(xe_tiles, E, sm_p, sc_p)
                for t in range(NT):
                    nc.scalar.activation(x_t[t][:], xe_tiles[t][:], AF.Copy,
                                         scale=r_emb[t][:])
                    nc.vector.tensor_copy(x0_t[t][:], x_t[t][:])

                # ---------------- layers ----------------
                for i in range(L_RUN):
                    qkvT_sb, cprojT_sb = [], []
                    for e in range(NE):
                        wtile = wq_p.tile([128, E], bf16, tag="wq")
                        nc.sync.dma_start(
                            wtile[:], d_qkvT[i, e * 128:(e + 1) * 128, :])
                        qkvT_sb.append(wtile)
                        ctile = wc_p.tile([128, E], bf16, tag="wc")
                        nc.sync.dma_start(
                            ctile[:], d_cprojT[i, e * 128:(e + 1) * 128, :])
                        cprojT_sb.append(ctile)

                    # residual blend: x = lamr*x + lamx*x0
                    for t in range(NT):
                        tmp = sc_p.tile([128, E], f32, tag="sc")
                        nc.scalar.activation(tmp[:], x0_t[t][:], AF.Copy,
                                             scale=lamx_t[:, i:i + 1])
                        nc.vector.scalar_tensor_tensor(
                            out=x_t[t][:], in0=x_t[t][:],
                            scalar=lamr_t[:, i:i + 1], in1=tmp[:],
                            op0=ALU.mult, op1=ALU.add)

                    # ---- attention input norm -> h (bf16) -> hT ----
                    r_at = rmsnorm_stats(x_t, E, sm_p, sc_p)
                    h_tiles = []
                    for t in range(NT):
                        h = hh_p.tile([128, E], bf16, tag="hh")
                        nc.scalar.activation(h[:], x_t[t][:], AF.Copy,
                                             scale=r_at[t][:])
                        h_tiles.append(h)
                    hT = []
                    for e in range(NE):
                        tp = psT.tile([128, TQ], bf16, tag="psT")
                        for t in range(NT):
                            nc.tensor.transpose(
                                tp[:, t * 128:(t + 1) * 128],
                                h_tiles[t][:, e * 128:(e + 1) * 128],
                                ident_bf[:])
                        hsb = ht_p.tile([128, TQ], bf16, tag="ht")
                        nc.vector.tensor_copy(hsb[:], tp[:])
                        hT.append(hsb)

                    # ---- qkv matmul + qk-norm + rope -> w_bf; ship to AG ----
                    cc_in = dpool.tile([TQ, E], bf16, tag="cc_in")
                    cc_out = dpool.tile([2 * TQ, E], bf16, tag="cc_out")
                    w_bf_tiles = []
                    for t in range(NT):
                        wps = [psB.tile([128, 384], f32, tag="psB", name="wps")
                               for _ in range(2)]
                        for ch in range(2):
                            for e in range(NE):
                                nc.tensor.matmul(
                                    wps[ch][:],
                                    hT[e][:, t * 128:(t + 1) * 128],
                                    qkvT_sb[e][:, ch * 384:(ch + 1) * 384],
                                    start=(e == 0), stop=(e == NE - 1))
                        # qk-norm stats on pre-rope w (rope is a rotation)
                        ssw = sm_p.tile([128, H], f32, tag="ssw")
                        for ch in range(2):
                            for hh in range(3):
                                hgl = ch * 3 + hh
                                junk = rp_p.tile([128, 128], f32, tag="sqj")
                                nc.scalar.activation(
                                    junk[:], wps[ch][:, hh * 128:(hh + 1) * 128],
                                    AF.Square,
                                    accum_out=ssw[:, hgl:hgl + 1])
                        sqw = sm_p.tile([128, H], f32, tag="sqw")
                        nc.scalar.activation(sqw[:], ssw[:], AF.Sqrt,
                                             bias=eps_t[:], scale=1.0 / HD)
                        rw = sm_p.tile([128, H], f32, tag="rw")
                        nc.vector.reciprocal(rw[:], sqw[:])
                        w_bf = wb_p.tile([128, E], bf16, tag="wb")
                        for ch in range(2):
                            wv = wps[ch][:].rearrange("p (h d) -> p h d", d=128)
                            x1 = wv[:, :, 0:64]
                            x2 = wv[:, :, 64:128]
                            cg = cosr_t[t][:, ch * 192:(ch + 1) * 192] \
                                .rearrange("p (h d) -> p h d", d=64)
                            sg = sinr_t[t][:, ch * 192:(ch + 1) * 192] \
                                .rearrange("p (h d) -> p h d", d=64)
                            t1 = rp_p.tile([128, 192], f32, tag="r1")
                            t2 = rp_p.tile([128, 192], f32, tag="r2")
                            t3 = rp_p.tile([128, 192], f32, tag="r3")
                            t4 = rp_p.tile([128, 192], f32, tag="r4")
                            v1 = t1[:].rearrange("p (h d) -> p h d", d=64)
                            v2 = t2[:].rearrange("p (h d) -> p h d", d=64)
                            v3 = t3[:].rearrange("p (h d) -> p h d", d=64)
                            v4 = t4[:].rearrange("p (h d) -> p h d", d=64)
                            nc.vector.tensor_mul(v1, x1, cg)
                            nc.vector.tensor_mul(v2, x2, sg)
                            nc.vector.tensor_mul(v3, x2, cg)
                            nc.vector.tensor_mul(v4, x1, sg)
                            wn = rp_p.tile([128, 384], f32, tag="wn")
                            wnv = wn[:].rearrange("p (h d) -> p h d", d=128)
                            nc.vector.tensor_add(wnv[:, :, 0:64], v1, v2)
                            nc.vector.tensor_sub(wnv[:, :, 64:128], v3, v4)
                            for hh in range(3):
                                hgl = ch * 3 + hh
                                nc.scalar.activation(
                                    w_bf[:, hgl * 128:(hgl + 1) * 128],
                                    wn[:, hh * 128:(hh + 1) * 128],
                                    AF.Copy, scale=rw[:, hgl:hgl + 1])
                        nc.sync.dma_start(
                            cc_in[t * 128:(t + 1) * 128, :], w_bf[:])
                        w_bf_tiles.append(w_bf)

                    # own queries, transposed per head: wTown[h] = [d, q]
                    wTown = []
                    for h in range(H):
                        tp = psT.tile([128, TQ], bf16, tag="psT")
                        for t in range(NT):
                            nc.tensor.transpose(
                                tp[:, t * 128:(t + 1) * 128],
                                w_bf_tiles[t][:, h * 128:(h + 1) * 128],
                                ident_bf[:])
                        wsb = wo_p.tile([128, TQ], bf16, tag="wo")
                        nc.vector.tensor_copy(wsb[:], tp[:])
                        wTown.append(wsb)

                    nc.gpsimd.collective_compute(
                        "AllGather", mybir.AluOpType.bypass,
                        replica_groups=groups,
                        ins=[cc_in[:]], outs=[cc_out[:]])

                    # ---- attention per head ----
                    oT_sb = []
                    for h in range(H):
                        wall = wl_p.tile([128, 8, 128], bf16, tag="wl")
                        nc.sync.dma_start(
                            wall[:],
                            cc_out[:, h * 128:(h + 1) * 128]
                            .rearrange("(n p) d -> p n d", p=128))
                        wT = psT.tile([128, 8, 128], bf16, tag="psT")
                        for kb in range(8):
                            nc.tensor.transpose(
                                wT[:, kb, :], wall[:, kb, :], ident_bf[:])
                        wTall = wt_p.tile([128, 2 * TQ], bf16, tag="wt")
                        nc.vector.tensor_copy(
                            wTall[:], wT[:].rearrange("p n d -> p (n d)"))

                        pT = pT_tiles[h % 2]
                        for lt in range(NT):
                            nk = (lt + 1) * 128
                            sps = [psA.tile([128, 512], f32, tag="psA", name="sps")
                                   for _ in range(2)]
                            for rnk in range(2):
                                nc.tensor.matmul(
                                    sps[rnk][:, 0:nk],
                                    wTown[h][:, lt * 128:(lt + 1) * 128],
                                    wTall[:, rnk * TQ:rnk * TQ + nk],
                                    start=True, stop=True)
                                nc.vector.tensor_add(
                                    sps[rnk][:, lt * 128:nk],
                                    sps[rnk][:, lt * 128:nk],
                                    dmask_t[:, rnk, :])
                            m0 = sm_p.tile([128, 1], f32, tag="m0")
                            m1 = sm_p.tile([128, 1], f32, tag="m1")
                            nc.vector.reduce_max(m0[:], sps[0][:, 0:nk],
                                                 axis=mybir.AxisListType.X)
                            nc.vector.reduce_max(m1[:], sps[1][:, 0:nk],
                                                 axis=mybir.AxisListType.X)
                            mm = sm_p.tile([128, 1], f32, tag="mm")
                            nc.vector.tensor_max(mm[:], m0[:], m1[:])
                            negm = sm_p.tile([128, 1], f32, tag="negm")
                            nc.vector.tensor_scalar_mul(negm[:], mm[:], -SCALE)
                            pr = pp_p.tile([128, 2, TQ], bf16, tag="pp")
                            se0 = sm_p.tile([128, 1], f32, tag="se0")
                            se1 = sm_p.tile([128, 1], f32, tag="se1")
                            for rnk, seb in ((0, se0), (1, se1)):
                                nc.scalar.activation(
                                    pr[:, rnk, 0:nk], sps[rnk][:, 0:nk],
                                    AF.Exp, bias=negm[:], scale=SCALE,
                                    accum_out=seb[:])
                            se = sm_p.tile([128, 1], f32, tag="se")
                            nc.vector.tensor_add(se[:], se0[:], se1[:])
                            rs = sm_p.tile([128, 1], f32, tag="rs")
                            nc.vector.reciprocal(rs[:], se[:])
                            for rnk in range(2):
                                nc.vector.tensor_scalar_mul(
                                    pr[:, rnk, 0:nk], pr[:, rnk, 0:nk], rs[:])
                            for rnk in range(2):
                                for j in range(lt + 1):
                                    tp = psT.tile([128, 128], bf16, tag="psT")
                                    nc.tensor.transpose(
                                        tp[:],
                                        pr[:, rnk, j * 128:(j + 1) * 128],
                                        ident_bf[:])
                                    nc.vector.tensor_copy(
                                        pT[:, rnk, j,
                                           lt * 128:(lt + 1) * 128], tp[:])
                        # AV: oT[d, q] accumulated over all (rank, j) blocks
                        ops = psA.tile([128, TQ], f32, tag="psA")
                        first = True
                        for rnk in range(2):
                            for j in range(NT):
                                nc.tensor.matmul(
                                    ops[:], wall[:, rnk * 4 + j, :],
                                    pT[:, rnk, j, :],
                                    start=first,
                                    stop=(rnk == 1 and j == NT - 1))
                                first = False
                        osb = ot_p.tile([128, TQ], bf16, tag="ot")
                        nc.vector.tensor_copy(osb[:], ops[:])
                        oT_sb.append(osb)

                    # ---- cproj + residual add ----
                    for t in range(NT):
                        for ch in range(2):
                            cps = psB.tile([128, 384], f32, tag="psB")
                            for e in range(NE):
                                nc.tensor.matmul(
                                    cps[:],
                                    oT_sb[e][:, t * 128:(t + 1) * 128],
                                    cprojT_sb[e][:, ch * 384:(ch + 1) * 384],
                                    start=(e == 0), stop=(e == NE - 1))
                            nc.vector.tensor_add(
                                x_t[t][:, ch * 384:(ch + 1) * 384],
                                x_t[t][:, ch * 384:(ch + 1) * 384],
                                cps[:])

                    # ---- ODL ----
                    r_od = rmsnorm_stats(x_t, E, sm_p, sc_p)
                    h2_tiles = []
                    for t in range(NT):
                        h2 = hh_p.tile([128, E], bf16, tag="hh")
                        nc.scalar.activation(h2[:], x_t[t][:], AF.Copy,
                                             scale=r_od[t][:])
                        h2_tiles.append(h2)
                    h2T = []
                    for e in range(NE):
                        tp = psT.tile([128, TQ], bf16, tag="psT")
                        for t in range(NT):
                            nc.tensor.transpose(
                                tp[:, t * 128:(t + 1) * 128],
                                h2_tiles[t][:, e * 128:(e + 1) * 128],
                                ident_bf[:])
                        hsb = ht_p.tile([128, TQ], bf16, tag="ht")
                        nc.vector.tensor_copy(hsb[:], tp[:])
                        h2T.append(hsb)

                    for jc in range(NJC):
                        dtiles = []
                        for e in range(NE):
                            dt_ = wd_p.tile([128, 512], bf16, tag="wd")
                            nc.sync.dma_start(
                                dt_[:],
                                d_dencT[i, e * 128:(e + 1) * 128,
                                        jc * 512:(jc + 1) * 512])
                            dtiles.append(dt_)
                        ddtiles = []
                        for jt in range(4):
                            ddt = wdd_p.tile([128, E], bf16, tag="wdd")
                            nc.sync.dma_start(
                                ddt[:],
                                d_ddecT[i, jc * 512 + jt * 128:
                                        jc * 512 + (jt + 1) * 128, :])
                            ddtiles.append(ddt)
                        aT = []
                        for jt in range(4):
                            aps = psA.tile([128, TQ], f32, tag="psA")
                            for e in range(NE):
                                nc.tensor.matmul(
                                    aps[:],
                                    dtiles[e][:, jt * 128:(jt + 1) * 128],
                                    h2T[e][:],
                                    start=(e == 0), stop=(e == NE - 1))
                            asb = at_p.tile([128, TQ], bf16, tag="at")
                            jgl = jc * 4 + jt
                            nc.scalar.activation(
                                asb[:], aps[:], AF.Relu,
                                bias=thrneg_t[:, i, jgl:jgl + 1])
                            aT.append(asb)
                        for t in range(NT):
                            for ch in range(2):
                                dps = psB.tile([128, 384], f32, tag="psB")
                                for jt in range(4):
                                    nc.tensor.matmul(
                                        dps[:],
                                        aT[jt][:, t * 128:(t + 1) * 128],
                                        ddtiles[jt][:, ch * 384:(ch + 1) * 384],
                                        start=(jt == 0), stop=(jt == 3))
                                nc.vector.tensor_add(
                                    x_t[t][:, ch * 384:(ch + 1) * 384],
                                    x_t[t][:, ch * 384:(ch + 1) * 384],
                                    dps[:])

                # ---------------- final rmsnorm -> hfT ----------------
                r_f = rmsnorm_stats(x_t, E, sm_p, sc_p)
                hf_tiles = []
                for t in range(NT):
                    hf = hh_p.tile([128, E], bf16, tag="hh")
                    nc.scalar.activation(hf[:], x_t[t][:], AF.Copy,
                                         scale=r_f[t][:])
                    hf_tiles.append(hf)
                for e in range(NE):
                    tp = psT.tile([128, TQ], bf16, tag="psT")
                    for t in range(NT):
                        nc.tensor.transpose(
                            tp[:, t * 128:(t + 1) * 128],
                            hf_tiles[t][:, e * 128:(e + 1) * 128],
                            ident_bf[:])
                    nc.vector.tensor_copy(hfT[e][:], tp[:])

            # ---------------- lm head ----------------
            with ExitStack() as mes:
                lmw_p = mes.enter_context(tc.tile_pool(name="lmw", bufs=12))
                lg_p = mes.enter_context(tc.tile_pool(name="lg", bufs=4))
                for vc, (vs, vw) in enumerate(VCH):
                    ltiles = []
                    for e in range(NE):
                        lw = lmw_p.tile([128, 512], bf16, tag="lmw")
                        nc.sync.dma_start(
                            lw[:, 0:vw],
                            d_lmT[e * 128:(e + 1) * 128, vs:vs + vw])
                        ltiles.append(lw)
                    for t in range(NT):
                        lps = psA.tile([128, 512], f32, tag="psA")
                        for e in range(NE):
                            nc.tensor.matmul(
                                lps[:, 0:vw],
                                hfT[e][:, t * 128:(t + 1) * 128],
                                ltiles[e][:, 0:vw],
                                start=(e == 0), stop=(e == NE - 1))
                        lg = lg_p.tile([128, 512], f32, tag="lg")
                        nc.vector.tensor_copy(lg[:, 0:vw], lps[:, 0:vw])
                        nc.sync.dma_start(
                            d_logits[t * 128:(t + 1) * 128, vs:vs + vw],
                            lg[:, 0:vw])

    nc.compile()
    return nc


def kernel(**inputs):
    global _NC_CACHE
    from concourse.bass_utils import run_bass_kernel_spmd

    prep, per_core = _host_prep(inputs)
    if _NC_CACHE is None:
        _NC_CACHE = _build_nc()
    nc = _NC_CACHE

    in_maps = []
    for c in range(N_CORES):
        pc = per_core[c]
        in_maps.append({
            "xemb": pc["xemb"], "cosr": pc["cosr"], "sinr": pc["sinr"],
            "dmask": pc["dmask"],
            "qkvT": prep["qkvT"], "cprojT": prep["cprojT"],
            "dencT": prep["dencT"], "ddecT": prep["ddecT"],
            "lmT": prep["lmT"], "thrneg": prep["thrneg"],
            "lamr": prep["lamr"], "lamx": prep["lamx"],
        })
    trace = bool(_os.environ.get("KBENCH_TRACE"))
    res = run_bass_kernel_spmd(nc, in_maps, core_ids=list(range(N_CORES)),
                               trace=trace,
                               trace_cores=list(range(N_CORES)) if trace else None)
    global LAST_RESULT
    LAST_RESULT = res
    out = np.empty((B, T, V), dtype=np.float32)
    for c in range(N_CORES):
        b, half = c // 2, c % 2
        out[b, _own_rows(half)] = res.results[c]["logits"]
    return out



# revision 16
# speedup vs baseline: 1.3312x; 1.3312x over previous
"""Trainium2 Bass kernel for nn_CRATE (12-layer CRATE-style transformer).

Sharding over 8 NeuronCores: 4 batch groups x 2-way parity-interleaved
sequence split.  Core c handles batch b=c//2 and parity half=c%2: it owns
absolute rows {2*j + half, j=0..511}.  With this split both halves have an
IDENTICAL causal block structure, so a single SPMD program serves all
cores; every half-dependence (rope phases, diagonal masks, embedding rows,
rank-select weights) is per-core input data.

v2 redesign vs the baseline:
  * scores computed TRANSPOSED ([k, q]) via the tied-QKV trick (queries
    are keys): no probability transposes at all (was 120/layer).
  * max-free softmax: qk-norm bounds |s|*scale <= sqrt(HD) ~ 11.3, so
    exp(s*scale) <= 8.2e4 fits fp32/bf16 comfortably.  Every (rank, block)
    becomes independent: own-rank scores+exp run DURING the AllGather.
  * softmax denominator via ones-column matmuls accumulated in PSUM;
    normalization applied at the attention output with a PE-broadcast
    reciprocal row.
  * rmsnorm scales deferred into matmul epilogues for the attention and
    lm-head paths.
  * the other rank's w is selected from the AllGather output with
    per-core 0/1 selector weights (one SPMD program, no dynamic DMA).
  * ODL decoder accumulates all 24 hidden blocks in PSUM: 8 residual
    adds per layer instead of 48.
  * lm head: one batched DMA per 512-vocab chunk for weights and one
    for logits.
Matmuls are bf16 with fp32 accumulation; residual stream and stats fp32.
"""

import sys

sys.path.insert(0, "/opt/trn_rl_repo")

import math
import numpy as np
import ml_dtypes

BF16 = ml_dtypes.bfloat16

B, T = 4, 1024
V, E, L, H = 50304, 768, 12, 6
HD = 128
HID = 3072
EPS = 1e-6
ROPE_BASE = 10000.0
SCALE = HD ** -0.5
N_CORES = 8
TQ = 512            # rows per core
NT = 4              # 128-row tiles per core
NE = 6              # 128-col tiles of E
NJT = 24            # 128-col tiles of HID
VCH = [(s, min(512, V - s)) for s in range(0, V, 512)]   # 99 vocab chunks
import os as _os
L_RUN = int(_os.environ.get("KBENCH_LAYERS", str(L)))


def _rope_tables():
    ch = np.arange(0, HD, 2, dtype=np.float32)
    inv = (1.0 / (ROPE_BASE ** (ch / np.float32(HD)))).astype(np.float32)
    t = np.arange(T, dtype=np.float32)
    fr = np.outer(t, inv).astype(np.float32)
    return np.cos(fr).astype(np.float32), np.sin(fr).astype(np.float32)


def _own_rows(half):
    return 2 * np.arange(TQ) + half


def _f32(a):
    return np.asarray(a, dtype=np.float32)


def _bf(a):
    return np.asarray(a).astype(BF16)


def _bfr(a):
    return np.asarray(a).astype(BF16).astype(np.float32)


def _mask_T(half):
    """maskT[k, s, q]: slot s=0 own-rank keys, s=1 other-rank keys.
    1.0 where query row (2q+half) >= key row (2k+r_key)."""
    ki = np.arange(128)[:, None]
    qi = np.arange(128)[None, :]
    out = np.empty((128, 2, 128), dtype=np.float32)
    for s, r in ((0, half), (1, 1 - half)):
        out[:, s, :] = (2 * qi + half >= 2 * ki + r).astype(np.float32)
    return out.astype(BF16)


def _host_prep(inputs):
    idx = np.asarray(inputs["idx"])
    wte = _f32(inputs["wte"])
    prep = {}
    prep["qkvT"] = np.ascontiguousarray(
        _f32(inputs["qkv_w"]).transpose(0, 2, 1)).astype(BF16)     # [L, E, E] (e, f)
    prep["cprojT"] = np.ascontiguousarray(
        _f32(inputs["cproj_w"]).transpose(0, 2, 1)).astype(BF16)   # [L, E, E] (e, e')
    prep["dencT"] = np.ascontiguousarray(
        _f32(inputs["denc_w"]).transpose(0, 2, 1)).astype(BF16)    # [L, E, HID]
    prep["ddecT"] = np.ascontiguousarray(
        _f32(inputs["ddec_w"]).transpose(0, 2, 1)).astype(BF16)    # [L, HID, E]
    prep["lmT"] = np.ascontiguousarray(_f32(inputs["lm_head_w"]).T).astype(BF16)
    thr = _f32(inputs["thr"])
    prep["thrneg"] = np.ascontiguousarray(
        (-thr).reshape(L, NJT, 128).transpose(2, 0, 1)).astype(np.float32)
    prep["lamr"] = np.ascontiguousarray(
        np.broadcast_to(_f32(inputs["resid_lambdas"]), (128, L))).astype(np.float32)
    prep["lamx"] = np.ascontiguousarray(
        np.broadcast_to(_f32(inputs["x0_lambdas"]), (128, L))).astype(np.float32)

    cos, sin = _rope_tables()          # [T, 64]
    per_core = []
    for c in range(N_CORES):
        b, half = c // 2, c % 2
        rows = _own_rows(half)
        pc = {}
        pc["xemb"] = np.ascontiguousarray(wte[idx[b][rows]]).astype(np.float32)
        pc["cosr"] = np.ascontiguousarray(np.tile(cos[rows], (1, H))).astype(BF16)
        pc["sinr"] = np.ascontiguousarray(np.tile(sin[rows], (1, H))).astype(BF16)
        pc["maskT"] = _mask_T(half)                       # [128, 2, 128] bf16
        # osel: wall_other = osel0 * wall_rank0 + osel1 * wall_rank1
        osel = np.zeros((128, 2), dtype=np.float32)
        osel[:, 1 - half] = 1.0
        pc["osel"] = osel
        per_core.append(pc)
    return prep, per_core


# --------------------------------------------------------------------------
# numpy mirror of the exact device dataflow (bf16 casts in the same places)
# --------------------------------------------------------------------------

def _mirror_pair(prep, pcs):
    xs = []
    for half in range(2):
        xe = pcs[half]["xemb"]
        xb = _bfr(xe)
        r = 1.0 / np.sqrt((xb * xb).sum(-1, keepdims=True) / E + EPS)
        xs.append((xe * r).astype(np.float32))
    x0s = [x.copy() for x in xs]

    for i in range(L_RUN):
        rl = prep["lamr"][0, i]
        xl = prep["lamx"][0, i]
        w_bfs = []
        for half in range(2):
            x = (xs[half] * rl + x0s[half] * xl).astype(np.float32)
            xs[half] = x
            xb = _bfr(x)
            r = 1.0 / np.sqrt((xb * xb).sum(-1, keepdims=True) / E + EPS)
            # qkv on raw (unnormalized) xb; evac psum to bf16
            wraw = _bfr(xb @ _bfr(prep["qkvT"][i]))        # [TQ, E]
            wh = wraw.reshape(TQ, H, HD)
            ssw = (wh * wh).sum(-1)                        # [TQ, H]
            rw = 1.0 / np.sqrt(ssw * (r * r / HD) + EPS)
            rrw = (r * rw).astype(np.float32)
            cosr = _bfr(pcs[half]["cosr"]).reshape(TQ, H, 64)
            sinr = _bfr(pcs[half]["sinr"]).reshape(TQ, H, 64)
            x1, x2 = wh[..., :64], wh[..., 64:]
            t1 = _bfr(x1 * cosr); t2 = _bfr(x2 * sinr)
            t3 = _bfr(x2 * cosr); t4 = _bfr(x1 * sinr)
            wn = np.concatenate([_bfr(t1 + t2), _bfr(t3 - t4)], axis=-1)
            w_bfs.append(_bf(wn * rrw[:, :, None]).reshape(TQ, E))

        new_xs = []
        for half in range(2):
            x = xs[half]
            maskT = _f32(pcs[half]["maskT"])               # [128, slot, 128]
            # slot 0 = own, slot 1 = other
            w_slot = [_bfr(w_bfs[half]).reshape(TQ, H, HD),
                      _bfr(w_bfs[1 - half]).reshape(TQ, H, HD)]
            o_all = np.zeros((H, HD, TQ), dtype=np.float32)
            for h in range(H):
                pT = np.zeros((2, NT, 128, TQ), dtype=np.float32)
                for s in range(2):
                    wk = w_slot[s][:, h]                   # [TQ, HD] keys
                    for j in range(NT):
                        q0 = j * 128
                        sT = wk[q0:q0 + 128] @ w_slot[0][:, h].T  # [128, TQ]
                        p = _bfr(np.exp(sT[:, q0:] * SCALE))
                        pT[s, j, :, q0:] = p
                        pT[s, j, :, q0:q0 + 128] = _bfr(
                            pT[s, j, :, q0:q0 + 128] * maskT[:, s, :])
                se = pT.reshape(2 * NT * 128, TQ).sum(0)   # [TQ]
                rs = (1.0 / se).astype(np.float32)
                o = np.zeros((HD, TQ), dtype=np.float32)
                for s in range(2):
                    for j in range(NT):
                        q0 = j * 128
                        o[:, q0:] += w_slot[s][j * 128:(j + 1) * 128, h].T @ \
                            pT[s, j, :, q0:]
                o_all[h] = _bfr(o * rs[None, :])
            o = o_all.transpose(2, 0, 1).reshape(TQ, E)
            x = x + o @ _bfr(prep["cprojT"][i])
            xb2 = _bfr(x)
            r2 = 1.0 / np.sqrt((xb2 * xb2).sum(-1, keepdims=True) / E + EPS)
            h2 = _bfr(x * r2)
            a_raw = h2 @ _bfr(prep["dencT"][i])
            thr_i = -prep["thrneg"][:, i, :].T.reshape(HID)
            aT = _bfr(np.maximum(a_raw - thr_i, 0.0))
            x = x + aT @ _bfr(prep["ddecT"][i])
            new_xs.append(x.astype(np.float32))
        xs = new_xs

    outs = []
    for half in range(2):
        x = xs[half]
        xb = _bfr(x)
        r = 1.0 / np.sqrt((xb * xb).sum(-1, keepdims=True) / E + EPS)
        outs.append((_bfr(xb) @ _bfr(prep["lmT"])) * r)
    return outs


def kernel_numpy(**inputs):
    prep, per_core = _host_prep(inputs)
    out = np.empty((B, T, V), dtype=np.float32)
    for b in range(B):
        logits = _mirror_pair(prep, per_core[2 * b:2 * b + 2])
        for half in range(2):
            out[b, _own_rows(half)] = logits[half]
    return out


# --------------------------------------------------------------------------
# Bass/Tile kernel
# --------------------------------------------------------------------------

_NC_CACHE = None
LAST_RESULT = None


def _build_nc():
    import concourse.bacc as bacc
    import concourse.mybir as mybir
    import concourse.tile as tile
    from concourse.masks import make_identity

    f32 = mybir.dt.float32
    bf16 = mybir.dt.bfloat16
    AF = mybir.ActivationFunctionType
    ALU = mybir.AluOpType

    nc = bacc.Bacc("TRN2", target_bir_lowering=False, debug=False,
                   num_devices=N_CORES)

    d_xemb = nc.dram_tensor("xemb", [TQ, E], f32, kind="ExternalInput")
    d_cosr = nc.dram_tensor("cosr", [TQ, H * 64], bf16, kind="ExternalInput")
    d_sinr = nc.dram_tensor("sinr", [TQ, H * 64], bf16, kind="ExternalInput")
    d_maskT = nc.dram_tensor("maskT", [128, 2, 128], bf16, kind="ExternalInput")
    d_osel = nc.dram_tensor("osel", [128, 2], f32, kind="ExternalInput")
    d_qkvT = nc.dram_tensor("qkvT", [L, E, E], bf16, kind="ExternalInput")
    d_cprojT = nc.dram_tensor("cprojT", [L, E, E], bf16, kind="ExternalInput")
    d_dencT = nc.dram_tensor("dencT", [L, E, HID], bf16, kind="ExternalInput")
    d_ddecT = nc.dram_tensor("ddecT", [L, HID, E], bf16, kind="ExternalInput")
    d_lmT = nc.dram_tensor("lmT", [E, V], bf16, kind="ExternalInput")
    d_thrneg = nc.dram_tensor("thrneg", [128, L, NJT], f32, kind="ExternalInput")
    d_lamr = nc.dram_tensor("lamr", [128, L], f32, kind="ExternalInput")
    d_lamx = nc.dram_tensor("lamx", [128, L], f32, kind="ExternalInput")
    d_logits = nc.dram_tensor("logits", [TQ, V], f32, kind="ExternalOutput")

    groups = [[0, 1], [2, 3], [4, 5], [6, 7]]
    RT_HD = 1.0 / math.sqrt(HD)

    from contextlib import ExitStack

    with tile.TileContext(nc) as tc, ExitStack() as es:
        if True:
            st = es.enter_context(tc.tile_pool(name="state", bufs=1))
            ptp = es.enter_context(tc.tile_pool(name="ptp", bufs=1))
            dpool = es.enter_context(tc.tile_pool(name="dram", bufs=2, space="DRAM"))
            # PSUM budget: psA 3 + psB 2 + psT 2 + psO 1 = 8 banks
            psA = es.enter_context(tc.tile_pool(name="psA", bufs=3, space="PSUM"))
            psB = es.enter_context(tc.tile_pool(name="psB", bufs=2, space="PSUM"))
            psT = es.enter_context(tc.tile_pool(name="psT", bufs=2, space="PSUM"))
            psO = es.enter_context(tc.tile_pool(name="psO", bufs=1, space="PSUM"))
            # ---------------- persistent state ----------------
            ident = st.tile([128, 128], f32, tag="ident")
            make_identity(nc, ident[:])
            ident_bf = st.tile([128, 128], bf16, tag="ident_bf")
            nc.vector.tensor_copy(ident_bf[:], ident[:])
            maskT_t = st.tile([128, 2, 128], bf16, tag="maskT")
            nc.sync.dma_start(maskT_t[:], d_maskT[:])
            osel_t = st.tile([128, 2], f32, tag="osel")
            nc.sync.dma_start(osel_t[:], d_osel[:])
            ones_col = st.tile([128, 1], bf16, tag="ones_col")
            nc.vector.memset(ones_col[:], 1.0)
            ones_row = st.tile([1, 128], f32, tag="ones_row")
            nc.vector.memset(ones_row[:], 1.0)

            eps_t = st.tile([128, 1], f32, tag="eps")
            nc.vector.memset(eps_t[:], EPS)
            lamr_t = st.tile([128, L], f32, tag="lamr")
            lamx_t = st.tile([128, L], f32, tag="lamx")
            thrneg_t = st.tile([128, L, NJT], f32, tag="thrneg")
            nc.sync.dma_start(lamr_t[:], d_lamr[:])
            nc.sync.dma_start(lamx_t[:], d_lamx[:])
            nc.sync.dma_start(thrneg_t[:], d_thrneg[:])

            x_t = [st.tile([128, E], f32, tag=f"x{t}", name=f"x_{t}") for t in range(NT)]
            x0_t = [st.tile([128, E], f32, tag=f"x0{t}", name=f"x0_{t}") for t in range(NT)]
            cosr_t = [st.tile([128, H * 64], bf16, tag=f"cos{t}", name=f"cosr_{t}") for t in range(NT)]
            sinr_t = [st.tile([128, H * 64], bf16, tag=f"sin{t}", name=f"sinr_{t}") for t in range(NT)]
            for t in range(NT):
                nc.sync.dma_start(cosr_t[t][:], d_cosr[t * 128:(t + 1) * 128, :])
                nc.sync.dma_start(sinr_t[t][:], d_sinr[t * 128:(t + 1) * 128, :])
            hfT = [st.tile([128, TQ], bf16, tag=f"hfT{e}", name=f"hfT_{e}") for e in range(NE)]
            r_f = st.tile([128, NT], f32, tag="r_f")

            # pT tiles: zeroed once; exp rewrites exactly the same valid
            # region every reuse, zeros elsewhere persist.
            pT_tiles = [ptp.tile([128, 2, NT, TQ], bf16, tag=f"pT{z}", name=f"pT_{z}")
                        for z in range(2)]
            for pt in pT_tiles:
                nc.vector.memset(pt[:], 0.0)

            def transpose_group(src_slices, dst_sb):
                """Transpose [128,128] bf16 slices into dst_sb (psT staging)."""
                tp = psT.tile([128, 128 * len(src_slices)], bf16, tag="psT")
                for k, sl in enumerate(src_slices):
                    nc.tensor.transpose(tp[:, k * 128:(k + 1) * 128], sl,
                                        ident_bf[:])
                nc.any.tensor_copy(dst_sb, tp[:])

            with ExitStack() as les:
                wq_p = les.enter_context(tc.tile_pool(name="wq", bufs=NE))
                wc_p = les.enter_context(tc.tile_pool(name="wc", bufs=NE))
                wd_p = les.enter_context(tc.tile_pool(name="wd", bufs=12))
                wdd_p = les.enter_context(tc.tile_pool(name="wdd", bufs=12))
                jk_p = les.enter_context(tc.tile_pool(name="jk", bufs=2))
                sm_p = les.enter_context(tc.tile_pool(name="sm", bufs=12))
                xb_p = les.enter_context(tc.tile_pool(name="xb", bufs=4))
                h2_p = les.enter_context(tc.tile_pool(name="h2", bufs=4))
                ht_p = les.enter_context(tc.tile_pool(name="ht", bufs=7))
                wr_p = les.enter_context(tc.tile_pool(name="wr", bufs=3))
                rp_p = les.enter_context(tc.tile_pool(name="rp", bufs=2))
                wb_p = les.enter_context(tc.tile_pool(name="wb", bufs=4))
                wo_p = les.enter_context(tc.tile_pool(name="wo", bufs=6))
                wt_p = les.enter_context(tc.tile_pool(name="wt", bufs=3))
                wl_p = les.enter_context(tc.tile_pool(name="wl", bufs=4))
                wm_p = les.enter_context(tc.tile_pool(name="wm", bufs=2))
                ws_p = les.enter_context(tc.tile_pool(name="ws", bufs=4))
                ot_p = les.enter_context(tc.tile_pool(name="ot", bufs=6))
                at_p = les.enter_context(tc.tile_pool(name="at", bufs=12))
                se_p = les.enter_context(tc.tile_pool(name="se", bufs=2))
                bc_p = les.enter_context(tc.tile_pool(name="bc", bufs=1))

                def rms_r(tiles_bf, dim):
                    """r = Rsqrt(sum(xb^2)/dim + eps) per tile, from bf16."""
                    rs = []
                    for t in range(NT):
                        junk = jk_p.tile([128, dim], f32, tag="jk")
                        ssq = sm_p.tile([128, 1], f32, tag="nss")
                        nc.scalar.activation(junk[:], tiles_bf[t][:], AF.Square,
                                             accum_out=ssq[:])
                        sq = sm_p.tile([128, 1], f32, tag="nsq")
                        nc.scalar.activation(sq[:], ssq[:], AF.Sqrt,
                                             bias=eps_t[:], scale=1.0 / dim)
                        r = sm_p.tile([128, 1], f32, tag="nr")
                        nc.vector.reciprocal(r[:], sq[:])
                        rs.append(r)
                    return rs

                # ---------------- embedding + initial rmsnorm ----------------
                xe_tiles, xeb_tiles = [], []
                for t in range(NT):
                    xe = jk_p.tile([128, E], f32, tag="xe")
                    nc.sync.dma_start(xe[:], d_xemb[t * 128:(t + 1) * 128, :])
                    xe_tiles.append(xe)
                    xeb = xb_p.tile([128, E], bf16, tag="xb")
                    nc.vector.tensor_copy(xeb[:], xe[:])
                    xeb_tiles.append(xeb)
                r_emb = rms_r(xeb_tiles, E)
                for t in range(NT):
                    nc.scalar.activation(x_t[t][:], xe_tiles[t][:], AF.Copy,
                                         scale=r_emb[t][:])
                    nc.vector.tensor_copy(x0_t[t][:], x_t[t][:])

                # ---------------- layers ----------------
                for i in range(L_RUN):
                    qkvT_sb, cprojT_sb = [], []
                    for e in range(NE):
                        wtile = wq_p.tile([128, E], bf16, tag="wq")
                        nc.sync.dma_start(
                            wtile[:], d_qkvT[i, e * 128:(e + 1) * 128, :])
                        qkvT_sb.append(wtile)
                        ctile = wc_p.tile([128, E], bf16, tag="wc")
                        nc.sync.dma_start(
                            ctile[:], d_cprojT[i, e * 128:(e + 1) * 128, :])
                        cprojT_sb.append(ctile)

                    # residual blend: x = lamr*x + lamx*x0
                    for t in range(NT):
                        tmp = jk_p.tile([128, E], f32, tag="tmp")
                        nc.scalar.activation(tmp[:], x0_t[t][:], AF.Copy,
                                             scale=lamx_t[:, i:i + 1])
                        nc.vector.scalar_tensor_tensor(
                            out=x_t[t][:], in0=x_t[t][:],
                            scalar=lamr_t[:, i:i + 1], in1=tmp[:],
                            op0=ALU.mult, op1=ALU.add)

                    # cast xb (bf16) + deferred-r stats
                    xb_tiles = []
                    for t in range(NT):
                        xb = xb_p.tile([128, E], bf16, tag="xb")
                        nc.vector.tensor_copy(xb[:], x_t[t][:])
                        xb_tiles.append(xb)
                    r_at = rms_r(xb_tiles, E)
                    r2h = []            # r^2/HD per tile
                    for t in range(NT):
                        s2 = sm_p.tile([128, 1], f32, tag="s2h")
                        nc.scalar.activation(s2[:], r_at[t][:], AF.Square,
                                             scale=RT_HD)
                        r2h.append(s2)

                    # xT transposes -> hT (raw x, bf16)
                    hT = []
                    for e in range(NE):
                        hsb = ht_p.tile([128, TQ], bf16, tag="ht")
                        transpose_group(
                            [xb_tiles[t][:, e * 128:(e + 1) * 128]
                             for t in range(NT)], hsb[:])
                        hT.append(hsb)

                    # ---- qkv matmul (raw) + deferred qk-norm + rope ----
                    cc_in = dpool.tile([TQ, E], bf16, tag="cc_in")
                    cc_out = dpool.tile([2 * TQ, E], bf16, tag="cc_out")
                    w_bf_tiles = []
                    for t in range(NT):
                        wraw = wr_p.tile([128, E], bf16, tag="wr")
                        for ch in range(2):
                            wps = psB.tile([128, 384], f32, tag="psB", name="wps")
                            for e in range(NE):
                                nc.tensor.matmul(
                                    wps[:],
                                    hT[e][:, t * 128:(t + 1) * 128],
                                    qkvT_sb[e][:, ch * 384:(ch + 1) * 384],
                                    start=(e == 0), stop=(e == NE - 1))
                            nc.vector.tensor_copy(
                                wraw[:, ch * 384:(ch + 1) * 384], wps[:])
                        # qk-norm stats on raw bf16 w; deferred r via
                        # scale = r^2/HD
                        ssw = sm_p.tile([128, H], f32, tag="ssw")
                        for hg in range(H):
                            junk = rp_p.tile([128, 128], f32, tag="sqj")
                            nc.scalar.activation(
                                junk[:], wraw[:, hg * 128:(hg + 1) * 128],
                                AF.Square, accum_out=ssw[:, hg:hg + 1])
                        sqw = sm_p.tile([128, H], f32, tag="sqw")
                        nc.scalar.activation(sqw[:], ssw[:], AF.Sqrt,
                                             bias=eps_t[:], scale=r2h[t][:])
                        rw = sm_p.tile([128, H], f32, tag="rw")
                        nc.vector.reciprocal(rw[:], sqw[:])
                        rrw = sm_p.tile([128, H], f32, tag="rrw")
                        nc.vector.tensor_scalar_mul(rrw[:], rw[:], r_at[t][:])
                        w_bf = wb_p.tile([128, E], bf16, tag="wb")
                        for ch in range(2):
                            wv = wraw[:, ch * 384:(ch + 1) * 384] \
                                .rearrange("p (h d) -> p h d", d=128)
                            x1 = wv[:, :, 0:64]
                            x2 = wv[:, :, 64:128]
                            cg = cosr_t[t][:, ch * 192:(ch + 1) * 192] \
                                .rearrange("p (h d) -> p h d", d=64)
                            sg = sinr_t[t][:, ch * 192:(ch + 1) * 192] \
                                .rearrange("p (h d) -> p h d", d=64)
                            t1 = rp_p.tile([128, 192], bf16, tag="r1")
                            t2 = rp_p.tile([128, 192], bf16, tag="r2")
                            t3 = rp_p.tile([128, 192], bf16, tag="r3")
                            t4 = rp_p.tile([128, 192], bf16, tag="r4")
                            v1 = t1[:].rearrange("p (h d) -> p h d", d=64)
                            v2 = t2[:].rearrange("p (h d) -> p h d", d=64)
                            v3 = t3[:].rearrange("p (h d) -> p h d", d=64)
                            v4 = t4[:].rearrange("p (h d) -> p h d", d=64)
                            nc.vector.tensor_mul(v1, x1, cg)
                            nc.vector.tensor_mul(v2, x2, sg)
                            nc.vector.tensor_mul(v3, x2, cg)
                            nc.vector.tensor_mul(v4, x1, sg)
                            wn = rp_p.tile([128, 384], bf16, tag="wn")
                            wnv = wn[:].rearrange("p (h d) -> p h d", d=128)
                            nc.vector.tensor_add(wnv[:, :, 0:64], v1, v2)
                            nc.vector.tensor_sub(wnv[:, :, 64:128], v3, v4)
                            for hh in range(3):
                                hgl = ch * 3 + hh
                                nc.vector.tensor_scalar_mul(
                                    w_bf[:, hgl * 128:(hgl + 1) * 128],
                                    wn[:, hh * 128:(hh + 1) * 128],
                                    rrw[:, hgl:hgl + 1])
                        nc.sync.dma_start(
                            cc_in[t * 128:(t + 1) * 128, :], w_bf[:])
                        w_bf_tiles.append(w_bf)

                    nc.gpsimd.collective_compute(
                        "AllGather", mybir.AluOpType.bypass,
                        replica_groups=groups,
                        ins=[cc_in[:]], outs=[cc_out[:]])

                    # own queries/keys, transposed per head: wTown[h] = [d, q]
                    wTown = []
                    for h in range(H):
                        wsb = wo_p.tile([128, TQ], bf16, tag="wo")
                        transpose_group(
                            [w_bf_tiles[t][:, h * 128:(h + 1) * 128]
                             for t in range(NT)], wsb[:])
                        wTown.append(wsb)

                    def score_block(h, wT_src, slot, j, pT):
                        q0 = j * 128
                        nv = TQ - q0
                        sps = psA.tile([128, TQ], f32, tag="psA", name="sps")
                        nc.tensor.matmul(
                            sps[:, 0:nv],
                            wT_src[:, j * 128:(j + 1) * 128],
                            wTown[h][:, q0:TQ],
                            start=True, stop=True)
                        nc.scalar.activation(
                            pT[:, slot, j, q0:TQ], sps[:, 0:nv],
                            AF.Exp, scale=SCALE)
                        nc.vector.tensor_mul(
                            pT[:, slot, j, q0:q0 + 128],
                            pT[:, slot, j, q0:q0 + 128],
                            maskT_t[:, slot, :])

                    # own-rank scores/exp/mask for all heads (overlaps AG)
                    for h in range(H):
                        pT = pT_tiles[h % 2]
                        for j in range(NT):
                            score_block(h, wTown[h][:], 0, j, pT[:])

                    # other-rank w: select from AllGather output
                    wall_sel = []
                    for j in range(NT):
                        w0 = wl_p.tile([128, E], bf16, tag="wl")
                        nc.sync.dma_start(
                            w0[:], cc_out[j * 128:(j + 1) * 128, :])
                        w1 = wl_p.tile([128, E], bf16, tag="wl")
                        nc.sync.dma_start(
                            w1[:], cc_out[TQ + j * 128:TQ + (j + 1) * 128, :])
                        tmp = wm_p.tile([128, E], bf16, tag="wtmp")
                        nc.vector.tensor_scalar_mul(tmp[:], w1[:],
                                                    osel_t[:, 1:2])
                        wsl = ws_p.tile([128, E], bf16, tag="wsel")
                        nc.vector.scalar_tensor_tensor(
                            out=wsl[:], in0=w0[:], scalar=osel_t[:, 0:1],
                            in1=tmp[:], op0=ALU.mult, op1=ALU.add)
                        wall_sel.append(wsl)

                    # ---- per-head: other scores, se, AV, normalize ----
                    oT_sb = []
                    for h in range(H):
                        pT = pT_tiles[h % 2]
                        wTo = wt_p.tile([128, TQ], bf16, tag="wt")
                        transpose_group(
                            [wall_sel[j][:, h * 128:(h + 1) * 128]
                             for j in range(NT)], wTo[:])
                        for j in range(NT):
                            score_block(h, wTo[:], 1, j, pT[:])
                        # se = sum_k pT -> [1, TQ] psum via ones-matmuls
                        sep = psA.tile([1, TQ], f32, tag="psA", name="sep")
                        first = True
                        for sl in range(2):
                            for j in range(NT):
                                q0 = j * 128
                                nc.tensor.matmul(
                                    sep[:, q0:TQ], ones_col[:],
                                    pT[:, sl, j, q0:TQ],
                                    start=first,
                                    stop=(sl == 1 and j == NT - 1))
                                first = False
                        rs = se_p.tile([1, TQ], f32, tag="rs")
                        nc.vector.reciprocal(rs[:], sep[:])
                        bcp = psA.tile([128, TQ], f32, tag="psA", name="bcp")
                        nc.tensor.matmul(bcp[:], ones_row[:], rs[:],
                                         start=True, stop=True)
                        bc = bc_p.tile([128, TQ], f32, tag="bc")
                        nc.any.tensor_copy(bc[:], bcp[:])
                        # AV: o[d, q] accumulated over (slot, j); own j=0
                        # first covers the full width for has_written.
                        ops = psO.tile([128, TQ], f32, tag="psO")
                        first = True
                        for sl in range(2):
                            for j in range(NT):
                                q0 = j * 128
                                src = (w_bf_tiles[j] if sl == 0
                                       else wall_sel[j])
                                nc.tensor.matmul(
                                    ops[:, q0:TQ],
                                    src[:, h * 128:(h + 1) * 128],
                                    pT[:, sl, j, q0:TQ],
                                    start=first,
                                    stop=(sl == 1 and j == NT - 1))
                                first = False
                        osb = ot_p.tile([128, TQ], bf16, tag="ot")
                        nc.vector.tensor_mul(osb[:], ops[:], bc[:])
                        oT_sb.append(osb)

                    # ---- cproj + residual add ----
                    for t in range(NT):
                        for ch in range(2):
                            cps = psB.tile([128, 384], f32, tag="psB")
                            for e in range(NE):
                                nc.tensor.matmul(
                                    cps[:],
                                    oT_sb[e][:, t * 128:(t + 1) * 128],
                                    cprojT_sb[e][:, ch * 384:(ch + 1) * 384],
                                    start=(e == 0), stop=(e == NE - 1))
                            nc.vector.tensor_add(
                                x_t[t][:, ch * 384:(ch + 1) * 384],
                                x_t[t][:, ch * 384:(ch + 1) * 384],
                                cps[:])

                    # ---- ODL ----
                    xb2_tiles = []
                    for t in range(NT):
                        xb2 = xb_p.tile([128, E], bf16, tag="xb")
                        nc.vector.tensor_copy(xb2[:], x_t[t][:])
                        xb2_tiles.append(xb2)
                    r_od = rms_r(xb2_tiles, E)
                    h2_tiles = []
                    for t in range(NT):
                        h2 = h2_p.tile([128, E], bf16, tag="h2")
                        nc.scalar.activation(h2[:], x_t[t][:], AF.Copy,
                                             scale=r_od[t][:])
                        h2_tiles.append(h2)
                    h2T = []
                    for e in range(NE):
                        hsb = ht_p.tile([128, TQ], bf16, tag="ht")
                        transpose_group(
                            [h2_tiles[t][:, e * 128:(e + 1) * 128]
                             for t in range(NT)], hsb[:])
                        h2T.append(hsb)

                    # encoder+decoder in two halves of 12 hidden blocks:
                    # accumulate 12 decoder matmuls per (t, ch) in PSUM.
                    for hf in range(2):
                        aT_sb = []
                        for jc in range(3 * hf, 3 * hf + 3):
                            dtiles = []
                            for e in range(NE):
                                dt_ = wd_p.tile([128, 512], bf16, tag="wd")
                                nc.sync.dma_start(
                                    dt_[:],
                                    d_dencT[i, e * 128:(e + 1) * 128,
                                            jc * 512:(jc + 1) * 512])
                                dtiles.append(dt_)
                            for jt in range(4):
                                aps = psA.tile([128, TQ], f32, tag="psA")
                                for e in range(NE):
                                    nc.tensor.matmul(
                                        aps[:],
                                        dtiles[e][:, jt * 128:(jt + 1) * 128],
                                        h2T[e][:],
                                        start=(e == 0), stop=(e == NE - 1))
                                asb = at_p.tile([128, TQ], bf16, tag="at")
                                jgl = jc * 4 + jt
                                nc.scalar.activation(
                                    asb[:], aps[:], AF.Relu,
                                    bias=thrneg_t[:, i, jgl:jgl + 1])
                                aT_sb.append(asb)
                        ddtiles = []
                        for jl in range(12):
                            jgl = 12 * hf + jl
                            ddt = wdd_p.tile([128, E], bf16, tag="wdd")
                            nc.sync.dma_start(
                                ddt[:],
                                d_ddecT[i, jgl * 128:(jgl + 1) * 128, :])
                            ddtiles.append(ddt)
                        for t in range(NT):
                            for ch in range(2):
                                dps = psB.tile([128, 384], f32, tag="psB")
                                for jl in range(12):
                                    nc.tensor.matmul(
                                        dps[:],
                                        aT_sb[jl][:, t * 128:(t + 1) * 128],
                                        ddtiles[jl][:, ch * 384:(ch + 1) * 384],
                                        start=(jl == 0), stop=(jl == 11))
                                nc.vector.tensor_add(
                                    x_t[t][:, ch * 384:(ch + 1) * 384],
                                    x_t[t][:, ch * 384:(ch + 1) * 384],
                                    dps[:])

                # ------------- final rmsnorm (deferred) -> hfT -------------
                xbf_tiles = []
                for t in range(NT):
                    xbf = xb_p.tile([128, E], bf16, tag="xb")
                    nc.vector.tensor_copy(xbf[:], x_t[t][:])
                    xbf_tiles.append(xbf)
                r_fin = rms_r(xbf_tiles, E)
                for t in range(NT):
                    nc.vector.tensor_copy(r_f[:, t:t + 1], r_fin[t][:])
                for e in range(NE):
                    transpose_group(
                        [xbf_tiles[t][:, e * 128:(e + 1) * 128]
                         for t in range(NT)], hfT[e][:])

            # ---------------- lm head ----------------
            with ExitStack() as mes:
                lmw_p = mes.enter_context(tc.tile_pool(name="lmw", bufs=3))
                lg_p = mes.enter_context(tc.tile_pool(name="lg", bufs=3))
                for vc, (vs, vw) in enumerate(VCH):
                    lw = lmw_p.tile([128, NE, 512], bf16, tag="lmw")
                    nc.sync.dma_start(
                        lw[:, :, 0:vw],
                        d_lmT[:, vs:vs + vw].rearrange(
                            "(e p) v -> p e v", p=128))
                    lg = lg_p.tile([128, NT, 512], f32, tag="lg")
                    for t in range(NT):
                        lps = psA.tile([128, 512], f32, tag="psA")
                        for e in range(NE):
                            nc.tensor.matmul(
                                lps[:, 0:vw],
                                hfT[e][:, t * 128:(t + 1) * 128],
                                lw[:, e, 0:vw],
                                start=(e == 0), stop=(e == NE - 1))
                        nc.vector.tensor_scalar_mul(
                            lg[:, t, 0:vw], lps[:, 0:vw], r_f[:, t:t + 1])
                    nc.sync.dma_start(
                        d_logits[:, vs:vs + vw].rearrange(
                            "(t p) v -> p t v", p=128),
                        lg[:, :, 0:vw])

    nc.compile()
    return nc


def kernel(**inputs):
    global _NC_CACHE
    from concourse.bass_utils import run_bass_kernel_spmd

    prep, per_core = _host_prep(inputs)
    if _NC_CACHE is None:
        _NC_CACHE = _build_nc()
    nc = _NC_CACHE

    in_maps = []
    for c in range(N_CORES):
        pc = per_core[c]
        in_maps.append({
            "xemb": pc["xemb"], "cosr": pc["cosr"], "sinr": pc["sinr"],
            "maskT": pc["maskT"], "osel": pc["osel"],
            "qkvT": prep["qkvT"], "cprojT": prep["cprojT"],
            "dencT": prep["dencT"], "ddecT": prep["ddecT"],
            "lmT": prep["lmT"], "thrneg": prep["thrneg"],
            "lamr": prep["lamr"], "lamx": prep["lamx"],
        })
    trace = bool(_os.environ.get("KBENCH_TRACE"))
    res = run_bass_kernel_spmd(nc, in_maps, core_ids=list(range(N_CORES)),
                               trace=trace,
                               trace_cores=list(range(N_CORES)) if trace else None)
    global LAST_RESULT
    LAST_RESULT = res
    out = np.empty((B, T, V), dtype=np.float32)
    for c in range(N_CORES):
        b, half = c // 2, c % 2
        out[b, _own_rows(half)] = res.results[c]["logits"]
    return out
